# revision 1
# baseline (speedup 1.0000x reference)
"""Bass program builders + host orchestration for the CustomGAT kernel.

Three SPMD launches on 8 cores:
  L1: pano GAT layer 0   (in: x_panoT, edges pp) -> p0' slices
  L2: pano GAT layer 1   (in: p0'T, edges pp)    -> p1' slices
  L3: translate conv + NullModel + closing MLP   -> final [1, 20*128] slices

Edge phase per core: dst-sorted edges packed in 128-edge tiles that never
cross 128-dst chunks; tiles grouped per (chunk, src-bank), bank-major
stream; per-tile selector matmul accumulates [wmsg|exp] into a psum run;
runs accumulate into an SBUF accumulator; finalize divides + bias.
Gathers via gpsimd.dma_gather (int16 idx, 512B rows).
"""
import numpy as np

import concourse.bass as bass
import concourse.bacc as bacc
import concourse.mybir as mybir
from concourse.tile import TileContext
from concourse.vector_clock import ScopedClock
from concourse import bass_utils

F32 = mybir.dt.float32
I16 = mybir.dt.int16
AF = mybir.ActivationFunctionType
OP = mybir.AluOpType

P = 128
N_CORES = 8
BANK = 32768
G = 24                 # max tiles per gather batch
S = 4                  # tiles per compute subgroup


# ---------------------------------------------------------------- drain patch
def _patched_drain_and_barrier(self, tick_clock, wait_clock):
    victim = self.nc.sync.nop(nofuse=True)
    wait_clock.add_sem_waits(victim.ins, ScopedClock({None: tick_clock.global_clock}))
    si = victim.ins.sync_info
    waits = list(si.on_wait) if si is not None and si.on_wait else []
    if si is not None and len(waits) > 1:
        si.on_wait = waits[:1]
        for w in waits[1:]:
            extra = self.nc.sync.nop(nofuse=True)
            esi = extra.ins.sync_info
            if esi is None:
                extra.ins.sync_info = mybir.SyncInfo(on_wait=[w], on_update=[])
            else:
                esi.on_wait = [w]
    self.nc.sync.drain()
    self.nc.all_engine_barrier()
    popped = self.nc._tile_sem_poison_stack.pop()
    assert popped is self._sem_poison
    self.nc.clear_and_free_semaphores(list(self.sems.allocated().values()))
    self.nc.all_engine_barrier()


TileContext._drain_and_barrier = _patched_drain_and_barrier


# ---------------------------------------------------------------- host: plan
class Plan:
    __slots__ = ('n_chunks', 'n_banks', 'tiles', 'batches', 'runs', 'T',
                 'acc_mode', 'n_table_rows', 'final_acc')

    def __init__(self, **kw):
        for k, v in kw.items():
            setattr(self, k, v)


def build_plan_and_streams(src, dst, n_chunks_per_core, n_table_rows):
    """Returns (plan, per_core_streams)."""
    src = np.asarray(src, np.int64)
    dst = np.asarray(dst, np.int64)
    order = np.argsort(dst, kind='stable')
    s_src = src[order]
    s_dst = dst[order]
    n_banks = int(np.ceil(n_table_rows / BANK))
    core_span = n_chunks_per_core * P

    counts = np.zeros((N_CORES, n_chunks_per_core, n_banks), np.int64)
    lists = [[None] * n_chunks_per_core for _ in range(N_CORES)]
    for c in range(N_CORES):
        lo = np.searchsorted(s_dst, c * core_span, side='left')
        hi = np.searchsorted(s_dst, (c + 1) * core_span, side='left')
        cs, cd = s_src[lo:hi], s_dst[lo:hi]
        k_arr = (cd - c * core_span) // P
        b_arr = cs // BANK
        key = k_arr * n_banks + b_arr
        o2 = np.argsort(key, kind='stable')
        cs, cd, key = cs[o2], cd[o2], key[o2]
        bounds = np.searchsorted(key, np.arange(n_chunks_per_core * n_banks + 1))
        for k in range(n_chunks_per_core):
            per_bank = []
            for m in range(n_banks):
                i0, i1 = bounds[k * n_banks + m], bounds[k * n_banks + m + 1]
                per_bank.append((cs[i0:i1], cd[i0:i1]))
                counts[c, k, m] = i1 - i0
            lists[c][k] = per_bank

    tiles_km = np.ceil(counts / P).astype(np.int64).max(axis=0)
    empty = tiles_km.sum(axis=1) == 0
    tiles_km[empty, 0] = 1

    tiles, runs, acc_mode = [], [], []
    seen = set()
    for m in range(n_banks):
        for k in range(n_chunks_per_core):
            tk = int(tiles_km[k, m])
            if tk == 0:
                continue
            for i in range(tk):
                tiles.append((k, m))
                runs.append((i == 0, i == tk - 1))
                if i == tk - 1:
                    acc_mode.append('copy' if k not in seen else 'add')
                else:
                    acc_mode.append(None)
            seen.add(k)
    T = len(tiles)

    batches = []
    t = 0
    while t < T:
        m = tiles[t][1]
        n = 1
        while t + n < T and tiles[t + n][1] == m and n < G:
            n += 1
        batches.append((m, t, n))
        t += n

    final_acc = [False] * T
    last_end = {}
    for t in range(T):
        if runs[t][1]:
            last_end[tiles[t][0]] = t
    for k, t in last_end.items():
        final_acc[t] = True
    plan = Plan(n_chunks=n_chunks_per_core, n_banks=n_banks, tiles=tiles,
                batches=batches, runs=runs, T=T, acc_mode=acc_mode,
                n_table_rows=n_table_rows, final_acc=final_acc)

    streams = []
    for c in range(N_CORES):
        esrc = np.zeros((T, P), np.int64)
        hrloc = np.zeros((T, P), np.int64)
        dstloc = np.full((T, P), -1.0, np.float32)
        t = 0
        for m in range(n_banks):
            for k in range(n_chunks_per_core):
                tk = int(tiles_km[k, m])
                if tk == 0:
                    continue
                es, ed = lists[c][k][m]
                ne = len(es)
                fe = np.zeros(tk * P, np.int64)
                fh = np.zeros(tk * P, np.int64)
                fd = np.full(tk * P, -1.0, np.float32)
                fe[:ne] = es
                fh[:ne] = ed - c * core_span
                fd[:ne] = (ed - (c * core_span + k * P)).astype(np.float32)
                esrc[t:t + tk] = fe.reshape(tk, P)
                hrloc[t:t + tk] = fh.reshape(tk, P)
                dstloc[t:t + tk] = fd.reshape(tk, P)
                t += tk
        streams.append(dict(esrc=esrc, hrloc=hrloc, dstloc=dstloc))
    return plan, streams


def wrap_idx16(flat_idx):
    """[T,128] -> [128, T*8] int16 dma_gather layout (16-wrap, x8 replicated)."""
    n = flat_idx.size
    x = flat_idx.reshape(n)
    w = np.zeros((16, n // 16), np.int16)
    pos = np.arange(n)
    w[pos % 16, pos // 16] = x.astype(np.int16)
    return np.tile(w, (8, 1))


def make_stream_inputs(stream, lamL, lamR):
    esrc, hrloc, dstloc = stream['esrc'], stream['hrloc'], stream['dstloc']
    hl_idx = wrap_idx16(esrc % BANK)
    hr_idx = wrap_idx16(hrloc)
    dstlocT = np.ascontiguousarray(dstloc.T)
    L = (lamL[esrc] + lamR[hrloc]).astype(np.float32)       # [T,128,2]
    LT = np.ascontiguousarray(L.transpose(1, 0, 2).reshape(P, -1))
    return dict(hl_idx=hl_idx, hr_idx=hr_idx, dstlocT=dstlocT, LT=LT)


# ---------------------------------------------------------- conv transforms
def conv_transform(Wl, bl, Wr, br, att, b):
    H, C = att.shape
    a = np.asarray(att, np.float64).reshape(-1)
    perm, widths = [], []
    for h in range(H):
        cols = np.arange(h * C, (h + 1) * C)
        pos = cols[a[cols] >= 0]
        neg = cols[a[cols] < 0]
        widths.append(len(pos))
        perm.extend(pos.tolist())
        perm.extend(neg.tolist())
    perm = np.array(perm, np.int64)
    A = np.maximum(0.8 * np.abs(a[perm]), 1e-12)

    def scale_cols(W, bvec):
        W = np.asarray(W, np.float64)
        bvec = np.asarray(bvec, np.float64)
        return ((W[:, perm] * A[None, :]).astype(np.float32),
                (bvec[perm] * A).astype(np.float32))

    Wl_s, bl_s = scale_cols(Wl, bl)
    Wr_s, br_s = scale_cols(Wr, br)
    # lambda projectors (true-space): lam = x @ Wlam + blam, per head, x0.2
    Wlam_l = np.stack([0.2 * (np.asarray(Wl, np.float64)[:, h * C:(h + 1) * C]
                              @ a[h * C:(h + 1) * C]) for h in range(H)], 1)
    blam_l = np.array([0.2 * (np.asarray(bl, np.float64)[h * C:(h + 1) * C]
                              @ a[h * C:(h + 1) * C]) for h in range(H)])
    Wlam_r = np.stack([0.2 * (np.asarray(Wr, np.float64)[:, h * C:(h + 1) * C]
                              @ a[h * C:(h + 1) * C]) for h in range(H)], 1)
    blam_r = np.array([0.2 * (np.asarray(br, np.float64)[h * C:(h + 1) * C]
                              @ a[h * C:(h + 1) * C]) for h in range(H)])
    bprime = (np.asarray(b, np.float64)[perm] * A).astype(np.float32)
    return dict(perm=perm, A=A, widths=widths, Wl=Wl_s, bl=bl_s, Wr=Wr_s, br=br_s,
                Wlam_l=Wlam_l, blam_l=blam_l, Wlam_r=Wlam_r, blam_r=blam_r,
                bprime=bprime)


def input_fixup(W, perm, A):
    """Row-fixup so W consumes stored p' (scaled+permuted) instead of p."""
    W = np.asarray(W, np.float64)
    return (W[perm, :] / A[:, None]).astype(np.float32)


def rep(v):
    """Replicate row vector across 128 partitions."""
    v = np.asarray(v, np.float32).reshape(1, -1)
    return np.ascontiguousarray(np.repeat(v, P, 0))


COLS_CONST = np.ascontiguousarray(
    np.repeat(np.arange(P, dtype=np.float32)[None, :], P, 0))


# ------------------------------------------------------------ device pieces
def _edge_phase(nc, tc, plan, hl_tabs, hr_table, hl_idx, hr_idx, dstlocT, LT,
                cols_sb, acc, widths, bprep_sb, out_cb):
    w0, w1 = widths
    ranges = [(0, w0), (64, 64 + w1), (w0, 64), (64 + w1, 128)]  # RP0,RP1,RN0,RN1
    with (
        tc.tile_pool(name='eidx', bufs=3) as idx_pool,
        tc.tile_pool(name='emsg', bufs=3) as msg_pool,
        tc.tile_pool(name='esg', bufs=4) as sg_pool,
        tc.tile_pool(name='erp', bufs=6, space='PSUM') as run_psum_pool,
        tc.tile_pool(name='fin', bufs=4) as fin_pool,
    ):
        cur_psum = [None]
        MB = 4 * G
        megas = []
        for (bank, t0, nt) in plan.batches:
            if megas and megas[-1][0] + megas[-1][1] == t0 and \
                    megas[-1][1] + nt <= MB:
                megas[-1] = (megas[-1][0], megas[-1][1] + nt,
                             megas[-1][2] + [(bank, t0, nt)])
            else:
                megas.append((t0, nt, [(bank, t0, nt)]))
        for (tm0, tmn, bl) in megas:
            hli = idx_pool.tile([P, MB * 8], I16, tag='hli')
            hri = idx_pool.tile([P, MB * 8], I16, tag='hri')
            dlo = idx_pool.tile([P, MB], F32, tag='dlo')
            ltt = idx_pool.tile([P, MB * 2], F32, tag='ltt')
            nc.sync.dma_start(out=hli[:, :tmn * 8],
                              in_=hl_idx[:, tm0 * 8:(tm0 + tmn) * 8])
            nc.sync.dma_start(out=hri[:, :tmn * 8],
                              in_=hr_idx[:, tm0 * 8:(tm0 + tmn) * 8])
            nc.sync.dma_start(out=dlo[:, :tmn], in_=dstlocT[:, tm0:tm0 + tmn])
            nc.sync.dma_start(out=ltt[:, :tmn * 2],
                              in_=LT[:, tm0 * 2:(tm0 + tmn) * 2])
            for (bank, t0, nt) in bl:
                r0 = t0 - tm0
                msg = msg_pool.tile([P, G * P], F32, tag='msg')
                hrg = msg_pool.tile([P, G * P], F32, tag='hrg')
                nc.gpsimd.dma_gather(
                    out_ap=msg[:, :nt * P].rearrange("p (t d) -> p t d", d=P),
                    in_ap=hl_tabs[bank][:, :],
                    idxs_ap=hli[:, r0 * 8:(r0 + nt) * 8],
                    num_idxs=nt * P, num_idxs_reg=nt * P,
                    elem_size=P, single_packet=False)
                nc.gpsimd.dma_gather(
                    out_ap=hrg[:, :nt * P].rearrange("p (t d) -> p t d", d=P),
                    in_ap=hr_table[:, :],
                    idxs_ap=hri[:, r0 * 8:(r0 + nt) * 8],
                    num_idxs=nt * P, num_idxs_reg=nt * P,
                    elem_size=P, single_packet=False)
                for s0 in range(0, nt, S):
                    ns = min(S, nt - s0)
                    q0 = r0 + s0
                    sel = sg_pool.tile([P, S * P], F32, tag='sel')
                    tsb = sg_pool.tile([P, S * P], F32, tag='tsb')
                    usb = sg_pool.tile([P, S * P], F32, tag='usb')
                    rhs = sg_pool.tile([P, S * 130], F32, tag='rhs')
                    red = sg_pool.tile([P, S * 4], F32, tag='red')
                    ssb = sg_pool.tile([P, S * 2], F32, tag='ssb')
                    m_sl = msg[:, s0 * P:(s0 + ns) * P]
                    h_sl = hrg[:, s0 * P:(s0 + ns) * P]
                    nc.vector.tensor_tensor(
                        out=sel[:, :ns * P].rearrange("p (j c) -> p j c", c=P),
                        in0=cols_sb[:].rearrange("p (o c) -> p o c", o=1)
                        .to_broadcast([P, ns, P]),
                        in1=dlo[:, q0:q0 + ns].rearrange("p (j o) -> p j o", o=1)
                        .to_broadcast([P, ns, P]),
                        op=OP.is_equal)
                    nc.vector.tensor_tensor(out=tsb[:, :ns * P], in0=m_sl,
                                            in1=h_sl, op=OP.add)
                    nc.scalar.activation(out=usb[:, :ns * P], in_=tsb[:, :ns * P],
                                         func=AF.Relu)
                    uv = usb[:, :ns * P].rearrange("p (j c) -> p j c", c=P)
                    rv = red[:, :ns * 4].rearrange("p (j f) -> p j f", f=4)
                    for ri, (c0, c1) in enumerate(ranges):
                        nc.vector.tensor_reduce(
                            out=rv[:, :, ri:ri + 1],
                            in_=uv[:, :, c0:c1],
                            axis=mybir.AxisListType.X, op=OP.add)
                    sv = ssb[:, :ns * 2].rearrange("p (j h) -> p j h", h=2)
                    lv = ltt[:, q0 * 2:(q0 + ns) * 2].rearrange(
                        "p (j h) -> p j h", h=2)
                    nc.vector.tensor_tensor(out=sv, in0=lv, in1=rv[:, :, 0:2],
                                            op=OP.add)
                    nc.vector.tensor_tensor(out=sv, in0=sv, in1=rv[:, :, 2:4],
                                            op=OP.subtract)
                    rview = rhs[:, :ns * 130].rearrange("p (j c) -> p j c", c=130)
                    nc.scalar.activation(out=rview[:, :, 128:130], in_=sv,
                                         func=AF.Exp)
                    nc.gpsimd.tensor_tensor(
                        out=rview[:, :, 0:128].rearrange(
                            "p j (h c) -> p j h c", c=64),
                        in0=m_sl.rearrange("p (j h c) -> p j h c", h=2, c=64),
                        in1=rview[:, :, 128:130].rearrange(
                            "p j (h o) -> p j h o", o=1)
                        .to_broadcast([P, ns, 2, 64]),
                        op=OP.mult)
                    for j in range(ns):
                        t_idx = t0 + s0 + j
                        run_start, run_end = plan.runs[t_idx]
                        if run_start:
                            cur_psum[0] = run_psum_pool.tile(
                                [P, 130], F32, tag='runp', name='runp')
                        nc.tensor.matmul(
                            out=cur_psum[0][:],
                            lhsT=sel[:, j * P:(j + 1) * P],
                            rhs=rhs[:, j * 130:(j + 1) * 130],
                            start=run_start, stop=run_end)
                        if run_end:
                            k = plan.tiles[t_idx][0]
                            a_sl = acc[:, k * 130:(k + 1) * 130]
                            if plan.acc_mode[t_idx] == 'copy':
                                nc.scalar.activation(out=a_sl,
                                                     in_=cur_psum[0][:],
                                                     func=AF.Copy)
                            else:
                                nc.vector.tensor_tensor(out=a_sl, in0=a_sl,
                                                        in1=cur_psum[0][:],
                                                        op=OP.add)
                            if plan.final_acc[t_idx]:
                                _finalize_chunk(nc, fin_pool, acc, k,
                                                bprep_sb, out_cb)


def _finalize_chunk(nc, fin_pool, acc, k, bprep_sb, out_cb):
    dadj = fin_pool.tile([P, 2], F32, tag='dadj', name='dadj')
    rec = fin_pool.tile([P, 2], F32, tag='rec', name='rec')
    res = fin_pool.tile([P, P], F32, tag='res', name='res')
    nc.vector.tensor_scalar_add(
        out=dadj[:], in0=acc[:, k * 130 + 128:k * 130 + 130], scalar1=1e-16)
    nc.vector.reciprocal(out=rec[:], in_=dadj[:])
    nc.vector.tensor_tensor(
        out=res[:].rearrange("p (h c) -> p h c", c=64),
        in0=acc[:, k * 130:k * 130 + 128].rearrange("p (h c) -> p h c", c=64),
        in1=rec[:].rearrange("p (h o) -> p h o", o=1).to_broadcast([P, 2, 64]),
        op=OP.mult)
    nc.vector.tensor_tensor(out=res[:], in0=res[:], in1=bprep_sb[:], op=OP.add)
    out_cb(k, res)


def _finalize(nc, tc, plan, acc, bprep_sb, out_cb):
    with tc.tile_pool(name='fin', bufs=4) as fin_pool:
        for k in range(plan.n_chunks):
            dadj = fin_pool.tile([P, 2], F32, tag='dadj')
            rec = fin_pool.tile([P, 2], F32, tag='rec')
            res = fin_pool.tile([P, P], F32, tag='res')
            nc.vector.tensor_scalar_add(
                out=dadj[:], in0=acc[:, k * 130 + 128:k * 130 + 130],
                scalar1=1e-16)
            nc.vector.reciprocal(out=rec[:], in_=dadj[:])
            nc.vector.tensor_tensor(
                out=res[:].rearrange("p (h c) -> p h c", c=64),
                in0=acc[:, k * 130:k * 130 + 128].rearrange(
                    "p (h c) -> p h c", c=64),
                in1=rec[:].rearrange("p (h o) -> p h o", o=1)
                .to_broadcast([P, 2, 64]),
                op=OP.mult)
            nc.vector.tensor_tensor(out=res[:], in0=res[:], in1=bprep_sb[:],
                                    op=OP.add)
            out_cb(k, res)


def build_pano_layer(plan, D_in, widths):
    nc = bacc.Bacc("TRN2", target_bir_lowering=False, debug=False,
                   num_devices=N_CORES)
    NK = plan.n_chunks
    NROWS = plan.n_table_rows
    T = plan.T
    xT = nc.dram_tensor('xT', [D_in, NROWS], F32, kind='ExternalInput')
    xTs = nc.dram_tensor('xTs', [D_in, NK * P], F32, kind='ExternalInput')
    Wl = nc.dram_tensor('Wl', [D_in, P], F32, kind='ExternalInput')
    Wr = nc.dram_tensor('Wr', [D_in, P], F32, kind='ExternalInput')
    blrep = nc.dram_tensor('blrep', [P, P], F32, kind='ExternalInput')
    brrep = nc.dram_tensor('brrep', [P, P], F32, kind='ExternalInput')
    bprep = nc.dram_tensor('bprep', [P, P], F32, kind='ExternalInput')
    colsc = nc.dram_tensor('colsc', [P, P], F32, kind='ExternalInput')
    hl_idx = nc.dram_tensor('hl_idx', [P, T * 8], I16, kind='ExternalInput')
    hr_idx = nc.dram_tensor('hr_idx', [P, T * 8], I16, kind='ExternalInput')
    dstlocT = nc.dram_tensor('dstlocT', [P, T], F32, kind='ExternalInput')
    LT = nc.dram_tensor('LT', [P, T * 2], F32, kind='ExternalInput')
    p_out = nc.dram_tensor('p_out', [NK * P, P], F32, kind='ExternalOutput')
    hl_tabs = [nc.dram_tensor(f'hl_table{m}',
                              [min(BANK, NROWS - m * BANK), P], F32,
                              kind='Internal')
               for m in range(plan.n_banks)]
    hr_table = nc.dram_tensor('hr_table', [NK * P, P], F32, kind='Internal')

    with TileContext(nc) as tc:
        with tc.tile_pool(name='const', bufs=1) as cpool:
            Wl_sb = cpool.tile([D_in, P], F32)
            Wr_sb = cpool.tile([D_in, P], F32)
            blrep_sb = cpool.tile([P, P], F32, tag='blrep')
            brrep_sb = cpool.tile([P, P], F32, tag='brrep')
            bprep_sb = cpool.tile([P, P], F32)
            cols_sb = cpool.tile([P, P], F32)
            acc = cpool.tile([P, NK * 130], F32)
            nc.sync.dma_start(out=Wl_sb[:], in_=Wl[:])
            nc.sync.dma_start(out=Wr_sb[:], in_=Wr[:])
            nc.sync.dma_start(out=blrep_sb[:], in_=blrep[:])
            nc.sync.dma_start(out=brrep_sb[:], in_=brrep[:])
            nc.sync.dma_start(out=bprep_sb[:], in_=bprep[:])
            nc.sync.dma_start(out=cols_sb[:], in_=colsc[:])

            with (
                tc.tile_pool(name='dps', bufs=3, space='PSUM') as psum_pool,
                tc.tile_pool(name='dstage', bufs=3) as stage_pool,
                tc.tile_pool(name='dxpage', bufs=3) as xpage_pool,
            ):
                _dense_table2(nc, tc, xTs, Wr_sb, hr_table, NK,
                              psum_pool, stage_pool, xpage_pool, brrep_sb)
                _dense_table2(nc, tc, xT, Wl_sb, hl_tabs, NROWS // P,
                              psum_pool, stage_pool, xpage_pool, blrep_sb)

            def emit(k, res):
                nc.sync.dma_start(out=p_out[k * P:(k + 1) * P, :], in_=res[:])
            _edge_phase(nc, tc, plan, hl_tabs, hr_table, hl_idx, hr_idx,
                        dstlocT, LT, cols_sb, acc, widths, bprep_sb, emit)
    nc.compile()
    return nc


def _dense_table2(nc, tc, xT, W_sb, table, n_tiles,
                  psum_pool, stage_pool, xpage_pool, brep_sb,
                  page_tiles=8):
    D = xT.shape[0]
    tabs = table if isinstance(table, list) else [table]
    if len(tabs) > 1:
        page_tiles = min(page_tiles, max(1, BANK // P))
    n_pages = (n_tiles + page_tiles - 1) // page_tiles
    for pg in range(n_pages):
        j0 = pg * page_tiles
        jn = min(page_tiles, n_tiles - j0)
        xp = xpage_pool.tile([D, page_tiles * P], F32, tag='xpage')
        nc.gpsimd.dma_start(out=xp[:, :jn * P], in_=xT[:, j0 * P:(j0 + jn) * P])
        stage = stage_pool.tile([P, page_tiles * P], F32, tag='stage')
        ps = psum_pool.tile([P, page_tiles * P], F32, tag='dps', name='dps')
        for j in range(jn):
            nc.tensor.matmul(out=ps[:, j * P:(j + 1) * P],
                             lhsT=xp[:, j * P:(j + 1) * P], rhs=W_sb[:],
                             start=True, stop=True)
        nc.vector.tensor_tensor(
            out=stage[:, :jn * P].rearrange("p (j c) -> p j c", c=P),
            in0=ps[:, :jn * P].rearrange("p (j c) -> p j c", c=P),
            in1=brep_sb[:].rearrange("p (o c) -> p o c", o=1)
            .to_broadcast([P, jn, P]),
            op=OP.add)
        r0 = j0 * P
        m = r0 // BANK
        lr = r0 - m * BANK
        nc.scalar.dma_start(
            out=tabs[m][lr:lr + jn * P, :].rearrange("(j p) c -> p j c", p=P),
            in_=stage[:, :jn * P].rearrange("p (j c) -> p j c", c=P))

def build_l3(plan, D_hl, widths):
    """Translate conv + NullModel + closing MLP. D_hl = 128 (p1' feats)."""
    nc = bacc.Bacc("TRN2", target_bir_lowering=False, debug=False,
                   num_devices=N_CORES)
    NK = plan.n_chunks           # 20
    NROWS = plan.n_table_rows    # 100352 (pano side)
    NFP = NK * P                 # 2560 local fp rows
    T = plan.T
    DF = 16
    xT = nc.dram_tensor('xT', [D_hl, NROWS], F32, kind='ExternalInput')      # p1'T
    fTs = nc.dram_tensor('fTs', [DF, NFP], F32, kind='ExternalInput')        # x_fpT slice
    Wl = nc.dram_tensor('Wl', [D_hl, P], F32, kind='ExternalInput')
    Wr = nc.dram_tensor('Wr', [DF, P], F32, kind='ExternalInput')
    blrep = nc.dram_tensor('blrep', [P, P], F32, kind='ExternalInput')
    brrep = nc.dram_tensor('brrep', [P, P], F32, kind='ExternalInput')
    bprep = nc.dram_tensor('bprep', [P, P], F32, kind='ExternalInput')
    colsc = nc.dram_tensor('colsc', [P, P], F32, kind='ExternalInput')
    ident = nc.dram_tensor('ident', [P, P], F32, kind='ExternalInput')
    hl_idx = nc.dram_tensor('hl_idx', [P, T * 8], I16, kind='ExternalInput')
    hr_idx = nc.dram_tensor('hr_idx', [P, T * 8], I16, kind='ExternalInput')
    dstlocT = nc.dram_tensor('dstlocT', [P, T], F32, kind='ExternalInput')
    LT = nc.dram_tensor('LT', [P, T * 2], F32, kind='ExternalInput')
    # MLP + NullModel weights
    mw1 = nc.dram_tensor('mw1', [P, 64], F32, kind='ExternalInput')   # input-fixed
    mb1 = nc.dram_tensor('mb1', [64, 1], F32, kind='ExternalInput')
    mw2 = nc.dram_tensor('mw2', [64, 64], F32, kind='ExternalInput')
    mb2 = nc.dram_tensor('mb2', [64, 1], F32, kind='ExternalInput')
    mw3 = nc.dram_tensor('mw3', [64, 1], F32, kind='ExternalInput')
    mb3 = nc.dram_tensor('mb3', [1, 1], F32, kind='ExternalInput')
    nsw = nc.dram_tensor('nsw', [DF, 64], F32, kind='ExternalInput')
    nsb = nc.dram_tensor('nsb', [64, 1], F32, kind='ExternalInput')
    nbw = nc.dram_tensor('nbw', [64, 64], F32, kind='ExternalInput')
    nbb = nc.dram_tensor('nbb', [64, 1], F32, kind='ExternalInput')
    ncw = nc.dram_tensor('ncw', [64, 1], F32, kind='ExternalInput')
    ncb = nc.dram_tensor('ncb', [1, 1], F32, kind='ExternalInput')
    nlw = nc.dram_tensor('nlw', [DF, 1], F32, kind='ExternalInput')
    nlb = nc.dram_tensor('nlb', [1, 1], F32, kind='ExternalInput')
    out = nc.dram_tensor('out', [1, NFP], F32, kind='ExternalOutput')
    hl_tabs = [nc.dram_tensor(f'hl_table{m}',
                              [min(BANK, NROWS - m * BANK), P], F32,
                              kind='Internal')
               for m in range(plan.n_banks)]
    hr_table = nc.dram_tensor('hr_table', [NFP, P], F32, kind='Internal')

    with TileContext(nc) as tc:
        with tc.tile_pool(name='const', bufs=1) as cpool:
            Wl_sb = cpool.tile([D_hl, P], F32)
            Wr_sb = cpool.tile([DF, P], F32)
            blrep_sb = cpool.tile([P, P], F32, tag='blrep')
            brrep_sb = cpool.tile([P, P], F32, tag='brrep')
            bprep_sb = cpool.tile([P, P], F32)
            cols_sb = cpool.tile([P, P], F32)
            id_sb = cpool.tile([P, P], F32)
            acc = cpool.tile([P, NK * 130], F32)
            fpT_sb = cpool.tile([P, NFP], F32)
            fT_sb = cpool.tile([DF, NFP], F32)
            sm = cpool.tile([P, 64 + 64 + 1 + 64 + 64 + 1 + 1], F32)  # packed small weights
            for dst_sb, src_d in ((Wl_sb, Wl), (Wr_sb, Wr),
                                  (bprep_sb, bprep), (cols_sb, colsc),
                                  (id_sb, ident), (fT_sb, fTs),
                                  (blrep_sb, blrep), (brrep_sb, brrep)):
                nc.sync.dma_start(out=dst_sb[:], in_=src_d[:])
            mw1_sb = cpool.tile([P, 64], F32)
            mw2_sb = cpool.tile([64, 64], F32)
            mw3_sb = cpool.tile([64, 1], F32)
            nsw_sb = cpool.tile([DF, 64], F32)
            nbw_sb = cpool.tile([64, 64], F32)
            ncw_sb = cpool.tile([64, 1], F32)
            nlw_sb = cpool.tile([DF, 1], F32)
            mb1_sb = cpool.tile([64, 1], F32)
            mb2_sb = cpool.tile([64, 1], F32)
            mb3_sb = cpool.tile([1, 1], F32)
            nsb_sb = cpool.tile([64, 1], F32)
            nbb_sb = cpool.tile([64, 1], F32)
            ncb_sb = cpool.tile([1, 1], F32)
            nlb_sb = cpool.tile([1, 1], F32)
            for dst_sb, src_d in ((mw1_sb, mw1), (mw2_sb, mw2), (mw3_sb, mw3),
                                  (nsw_sb, nsw), (nbw_sb, nbw), (ncw_sb, ncw),
                                  (nlw_sb, nlw), (mb1_sb, mb1), (mb2_sb, mb2),
                                  (mb3_sb, mb3), (nsb_sb, nsb), (nbb_sb, nbb),
                                  (ncb_sb, ncb), (nlb_sb, nlb)):
                nc.sync.dma_start(out=dst_sb[:], in_=src_d[:])

            with (
                tc.tile_pool(name='dps', bufs=3, space='PSUM') as psum_pool,
                tc.tile_pool(name='dstage', bufs=3) as stage_pool,
                tc.tile_pool(name='dxpage', bufs=3) as xpage_pool,
            ):
                _dense_table2(nc, tc, fTs, Wr_sb, hr_table, NK,
                              psum_pool, stage_pool, xpage_pool, brrep_sb)
                _dense_table2(nc, tc, xT, Wl_sb, hl_tabs, NROWS // P,
                              psum_pool, stage_pool, xpage_pool, blrep_sb)

            with tc.tile_pool(name='tps', bufs=2, space='PSUM') as tpsum_pool:
                def emit(k, res):
                    tp = tpsum_pool.tile([P, P], F32, tag='tp', name='tp')
                    nc.tensor.transpose(out=tp[:], in_=res[:], identity=id_sb[:])
                    nc.scalar.activation(out=fpT_sb[:, k * P:(k + 1) * P],
                                         in_=tp[:], func=AF.Copy)
                _edge_phase(nc, tc, plan, hl_tabs, hr_table, hl_idx, hr_idx,
                            dstlocT, LT, cols_sb, acc, widths, bprep_sb, emit)

            # MLP + NullModel (transposed layout; pages of 512 cols)
            with (
                tc.tile_pool(name='mps', bufs=4, space='PSUM') as mpsum,
                tc.tile_pool(name='msb', bufs=1) as msb,
            ):
                h1 = msb.tile([64, NFP], F32)
                h2 = msb.tile([64, NFP], F32)
                tot = msb.tile([1, NFP], F32)
                tmp1 = msb.tile([1, NFP], F32)
                PW = min(512, NFP)
                NPG = (NFP + PW - 1) // PW
                def _sl(pg):
                    return slice(pg * PW, min((pg + 1) * PW, NFP))
                for pg in range(NPG):
                    sl = _sl(pg)
                    w = sl.stop - sl.start
                    ps = mpsum.tile([64, PW], F32, tag='m64')
                    nc.tensor.matmul(out=ps[:, :w], lhsT=mw1_sb[:], rhs=fpT_sb[:, sl],
                                     start=True, stop=True)
                    nc.scalar.activation(out=h1[:, sl], in_=ps[:, :w], func=AF.Relu,
                                         bias=mb1_sb[:, 0:1])
                for pg in range(NPG):
                    sl = _sl(pg)
                    w = sl.stop - sl.start
                    ps = mpsum.tile([64, PW], F32, tag='m64')
                    nc.tensor.matmul(out=ps[:, :w], lhsT=mw2_sb[:], rhs=h1[:, sl],
                                     start=True, stop=True)
                    nc.scalar.activation(out=h2[:, sl], in_=ps[:, :w], func=AF.Relu,
                                         bias=mb2_sb[:, 0:1])
                for pg in range(NPG):
                    sl = _sl(pg)
                    w = sl.stop - sl.start
                    ps = mpsum.tile([1, PW], F32, tag='m1')
                    nc.tensor.matmul(out=ps[:, :w], lhsT=mw3_sb[:], rhs=h2[:, sl],
                                     start=True, stop=True)
                    nc.scalar.activation(out=tot[:, sl], in_=ps[:, :w], func=AF.Identity,
                                         bias=mb3_sb[:, 0:1])
                # NullModel
                for pg in range(NPG):
                    sl = _sl(pg)
                    w = sl.stop - sl.start
                    ps = mpsum.tile([64, PW], F32, tag='m64')
                    nc.tensor.matmul(out=ps[:, :w], lhsT=nsw_sb[:], rhs=fT_sb[:, sl],
                                     start=True, stop=True)
                    nc.scalar.activation(out=h1[:, sl], in_=ps[:, :w], func=AF.Relu,
                                         bias=nsb_sb[:, 0:1])
                for rep_i, (wsb, bsb) in enumerate(((nbw_sb, nbb_sb),
                                                    (nbw_sb, nbb_sb))):
                    src = h1 if rep_i == 0 else h2
                    dst = h2 if rep_i == 0 else h1
                    for pg in range(NPG):
                        sl = _sl(pg)
                        w = sl.stop - sl.start
                        ps = mpsum.tile([64, PW], F32, tag='m64')
                        nc.tensor.matmul(out=ps[:, :w], lhsT=wsb[:], rhs=src[:, sl],
                                         start=True, stop=True)
                        nc.scalar.activation(out=dst[:, sl], in_=ps[:, :w],
                                             func=AF.Relu, bias=bsb[:, 0:1])
                for pg in range(NPG):
                    sl = _sl(pg)
                    w = sl.stop - sl.start
                    ps = mpsum.tile([1, PW], F32, tag='m1')
                    nc.tensor.matmul(out=ps[:, :w], lhsT=ncw_sb[:], rhs=h1[:, sl],
                                     start=True, stop=True)
                    nc.scalar.activation(out=tmp1[:, sl], in_=ps[:, :w], func=AF.Identity,
                                         bias=ncb_sb[:, 0:1])
                nc.vector.tensor_tensor(out=tot[:], in0=tot[:], in1=tmp1[:],
                                        op=OP.add)
                for pg in range(NPG):
                    sl = _sl(pg)
                    w = sl.stop - sl.start
                    ps = mpsum.tile([1, PW], F32, tag='m1')
                    nc.tensor.matmul(out=ps[:, :w], lhsT=nlw_sb[:], rhs=fT_sb[:, sl],
                                     start=True, stop=True)
                    nc.scalar.activation(out=tmp1[:, sl], in_=ps[:, :w], func=AF.Identity,
                                         bias=nlb_sb[:, 0:1])
                nc.vector.tensor_tensor(out=tot[:], in0=tot[:], in1=tmp1[:],
                                        op=OP.add)
                nc.sync.dma_start(out=out[:], in_=tot[:])
    nc.compile()
    return nc


# ------------------------------------------------------------- host sim/orch
def sim_core(plan, stream, hl_tab, hr_tab, lamL, lamR, widths, bprime):
    """Numpy emulation of one core's edge phase + finalize (device-faithful)."""
    esrc, hrloc, dstloc = stream['esrc'], stream['hrloc'], stream['dstloc']
    w0, w1 = widths
    NK = plan.n_chunks
    acc = np.zeros((NK, P, 130), np.float32)
    L = (lamL[esrc] + lamR[hrloc]).astype(np.float32)
    for t, (k, m) in enumerate(plan.tiles):
        msg = hl_tab[esrc[t]].astype(np.float32)
        hrr = hr_tab[hrloc[t]].astype(np.float32)
        tt = msg + hrr
        u = np.maximum(tt, 0.0)
        RP0 = u[:, :w0].sum(1)
        RN0 = u[:, w0:64].sum(1)
        RP1 = u[:, 64:64 + w1].sum(1)
        RN1 = u[:, 64 + w1:].sum(1)
        s = np.stack([L[t, :, 0] + RP0 - RN0, L[t, :, 1] + RP1 - RN1], 1)
        e = np.exp(s).astype(np.float32)
        sel = (dstloc[t][:, None] == np.arange(P)[None, :]).astype(np.float32)
        rhs = np.concatenate([msg[:, :64] * e[:, 0:1], msg[:, 64:] * e[:, 1:2], e], 1)
        acc[k] += sel.T @ rhs
    num = acc[:, :, :128]
    den = acc[:, :, 128:130] + 1e-16
    res = np.concatenate([num[:, :, :64] / den[:, :, 0:1],
                          num[:, :, 64:] / den[:, :, 1:2]], 2)
    return (res + bprime[None, None, :]).reshape(NK * P, P).astype(np.float32)


def host_prepare(inp):
    """All host-side preprocessing independent of intermediate results."""
    f = {k: np.asarray(v) for k, v in inp.items()}
    c0 = conv_transform(f['c0_Wl'], f['c0_bl'], f['c0_Wr'], f['c0_br'],
                        f['c0_att'], f['c0_b'])
    c1 = conv_transform(f['c1_Wl'], f['c1_bl'], f['c1_Wr'], f['c1_br'],
                        f['c1_att'], f['c1_b'])
    ct = conv_transform(f['ct_Wl'], f['ct_bl'], f['ct_Wr'], f['ct_br'],
                        f['ct_att'], f['ct_b'])
    plan_pp, str_pp = build_plan_and_streams(f['epp_src'], f['epp_dst'], 98, 100352)
    plan_pf, str_pf = build_plan_and_streams(f['epf_src'], f['epf_dst'], 20, 100352)
    NPAD, FPAD = 100352, 20480
    x_pano = np.zeros((NPAD, 64), np.float32)
    x_pano[:f['x_pano'].shape[0]] = f['x_pano']
    x_fp = np.zeros((FPAD, 16), np.float32)
    x_fp[:f['x_fp'].shape[0]] = f['x_fp']
    return dict(f=f, c0=c0, c1=c1, ct=ct, plan_pp=plan_pp, str_pp=str_pp,
                plan_pf=plan_pf, str_pf=str_pf, x_pano=x_pano, x_fp=x_fp)


def layer_inputs(plan, streams, xT_full, x_slices, Wl_s, bl_s, Wr_s, br_s,
                 bprime, lamL, lamR_full):
    """Build the 8 per-core in_maps for a pano layer launch."""
    core_span = plan.n_chunks * P
    in_maps = []
    blrep_a, brrep_a = rep(bl_s), rep(br_s)
    bprep = rep(bprime)
    for c in range(N_CORES):
        st = make_stream_inputs(streams[c],
                                lamL, lamR_full[c * core_span:(c + 1) * core_span])
        in_maps.append(dict(
            xT=xT_full, xTs=x_slices[c], Wl=Wl_s, Wr=Wr_s,
            blrep=blrep_a, brrep=brrep_a,
            bprep=bprep, colsc=COLS_CONST, **st))
    return in_maps


def lam_of(x, Wlam, blam):
    return (x.astype(np.float64) @ Wlam + blam[None, :]).astype(np.float32)


def _ascontig(a):
    return np.ascontiguousarray(a, dtype=np.float32)


def run_model(inp, run_fn=None, trace=False):
    """Full 3-launch execution. run_fn(nc, in_maps) -> list of result dicts."""
    if run_fn is None:
        def run_fn(nc, in_maps):
            return bass_utils.run_bass_kernel_spmd(
                nc, in_maps, core_ids=list(range(N_CORES)), trace=trace).results
    pre = host_prepare(inp)
    f, c0, c1, ct = pre['f'], pre['c0'], pre['c1'], pre['ct']
    plan_pp, str_pp = pre['plan_pp'], pre['str_pp']
    plan_pf, str_pf = pre['plan_pf'], pre['str_pf']
    x = pre['x_pano']            # [100352, 64]
    x_fp = pre['x_fp']           # [20480, 16]
    span = 98 * P

    # ---- L1 ----
    xT = _ascontig(x.T)
    x_slices = [_ascontig(x[c * span:(c + 1) * span].T) for c in range(N_CORES)]
    lamL0 = lam_of(x, c0['Wlam_l'], c0['blam_l'])
    lamR0 = lam_of(x, c0['Wlam_r'], c0['blam_r'])
    nc1 = build_pano_layer(plan_pp, 64, c0['widths'])
    im1 = layer_inputs(plan_pp, str_pp, xT, x_slices, c0['Wl'], c0['bl'],
                       c0['Wr'], c0['br'], c0['bprime'], lamL0, lamR0)
    r1 = run_fn(nc1, im1)
    p0 = np.concatenate([r1[c]['p_out'] for c in range(N_CORES)], 0)  # [100352,128]

    # ---- L2 ----
    def rowfix(W):
        return (np.asarray(W, np.float64)[c0['perm'], :]
                / c0['A'][:, None]).astype(np.float32)
    W1l, W1r = rowfix(c1['Wl']), rowfix(c1['Wr'])
    Wlam1_l, Wlam1_r = rowfix(c1['Wlam_l']), rowfix(c1['Wlam_r'])
    lamL1 = lam_of(p0, Wlam1_l, c1['blam_l'])
    lamR1 = lam_of(p0, Wlam1_r, c1['blam_r'])
    p0T = _ascontig(p0.T)
    p0_slices = [_ascontig(r1[c]['p_out'].T) for c in range(N_CORES)]
    nc2 = build_pano_layer(plan_pp, 128, c1['widths'])
    im2 = layer_inputs(plan_pp, str_pp, p0T, p0_slices, W1l, c1['bl'],
                       W1r, c1['br'], c1['bprime'], lamL1, lamR1)
    r2 = run_fn(nc2, im2)
    p1 = np.concatenate([r2[c]['p_out'] for c in range(N_CORES)], 0)

    # ---- L3 ----
    def rowfix1(W):
        return (np.asarray(W, np.float64)[c1['perm'], :]
                / c1['A'][:, None]).astype(np.float32)
    Wtl = rowfix1(ct['Wl'])
    Wlamt_l = rowfix1(ct['Wlam_l'])
    lamLt = lam_of(p1, Wlamt_l, ct['blam_l'])
    lamRt = lam_of(x_fp, ct['Wlam_r'], ct['blam_r'])
    mw1f = input_fixup(f['m_w1'], ct['perm'], ct['A'])
    p1T = _ascontig(p1.T)
    fspan = 20 * P
    col = lambda v: _ascontig(np.asarray(v, np.float32).reshape(-1, 1))
    nc3 = build_l3(plan_pf, 128, ct['widths'])
    im3 = []
    for c in range(N_CORES):
        st = make_stream_inputs(str_pf[c], lamLt,
                                lamRt[c * fspan:(c + 1) * fspan])
        im3.append(dict(
            xT=p1T, fTs=_ascontig(x_fp[c * fspan:(c + 1) * fspan].T),
            Wl=Wtl, Wr=ct['Wr'],
            blrep=rep(ct['bl']), brrep=rep(ct['br']),
            bprep=rep(ct['bprime']), colsc=COLS_CONST,
            ident=np.eye(P, dtype=np.float32),
            mw1=mw1f, mb1=col(f['m_b1']), mw2=_ascontig(f['m_w2']),
            mb2=col(f['m_b2']), mw3=_ascontig(f['m_w3']), mb3=col(f['m_b3']),
            nsw=_ascontig(f['nm_sw']), nsb=col(f['nm_sb']),
            nbw=_ascontig(f['nm_bw']), nbb=col(f['nm_bb']),
            ncw=_ascontig(f['nm_cw']), ncb=col(f['nm_cb']),
            nlw=_ascontig(f['nm_lw']), nlb=col(f['nm_lb']), **st))
    r3 = run_fn(nc3, im3)
    out = np.concatenate([r3[c]['out'][0] for c in range(N_CORES)])
    return out[:20000].reshape(20000, 1).astype(np.float32)


# ---------------------------------------------------------------- kernel API
def kernel(**inputs):
    """Self-contained entry: full inputs -> full [20000, 1] float32 output."""
    return run_model(inputs)

